# revision 23
# baseline (speedup 1.0000x reference)
"""Trainium2 Bass kernel for nn_ExpandingLinear.

Reference computation:
    x_exp = concat([x, x[:, p0] * v0, x_exp1[:, p1] * v1], axis=1)   # [B, 2176]
    W     = scatter_add(weight_vals at [weight_rows, weight_cols])    # [2048, 2176]
    b     = scatter_add(bias_vals at bias_idx)                        # [2048]
    out   = x_exp @ W.T + b                                           # [B, 2048]

Every expanded feature c is a_c * x[:, q_c] for a resolvable (q_c, a_c)
(parent chains only reference earlier features), so the embed columns fold
into the base weight on the host:
    W_eff[o, q_c] += a_c * W[o, 2048 + c]      ->  out = x @ W_eff.T + b
which reduces the device work to a dense [1024, 2048] @ [2048, 2048]
matmul + bias per core (data-parallel batch shard, 8 cores).

The classic dense schedule is issue-rate-bound on the PE: 512 matmuls of
[128,128]@[128,512] at 1 col/cycle = 109.2us, measured gapless.  To go
below that floor this kernel uses ONE-LEVEL STRASSEN on the per-core
matmul C[1024,2048] = x[1024,2048] @ Wt[2048,2048]:

    A11..A22 = 2x2 blocks of x   ([512,1024] each)
    B11..B22 = 2x2 blocks of Wt  ([1024,1024] each)
    M1 = (A11+A22)(B11+B22)  M2 = (A21+A22)B11   M3 = A11(B12-B22)
    M4 = A22(B21-B11)        M5 = (A11+A12)B22   M6 = (A21-A11)(B11+B12)
    M7 = (A12-A22)(B21+B22)
    C11 = M1+M4-M5+M7   C12 = M3+M5   C21 = M2+M4   C22 = M1-M2+M3+M6

7 products of [512,1024]@[1024,1024] = 7 * 64 matmuls = 95.6us PE stream
(12.5% less).  The 7 B-combinations are formed ON THE HOST (free) and
streamed; the 5 A-combinations are formed on the vector engine from the
resident x (bf16 adds, hidden under the PE stream).  M-terms accumulate in
PSUM (8 banks = the 8 concurrent groups of a term); GPSIMD cannot read
PSUM, so every psum-consuming op runs on DVE.

Numerics: one Strassen level grows bf16 error only slightly (measured
5.4e-3 vs 3.4e-3 classic, gate 2e-2).  PSUM accumulates fp32; C combines
are fp32.

Device schedule (per core), ~127.4us HW exec (classic dense: 128.3us):
  - PE p-state warmup (junk matmuls) burns the HAM 0.65->2.4GHz ramp while
    the first input tiles are in flight; 128-partition ring-warmup reads
    absorb each DMA queue's per-engine descriptor cold start.
  - term order M3, M4, M2, M5, M1, M6, M7.  M3 (lhsT = raw A11) and M4
    (raw A22) run k-outer paced by the live x/B streams, step-inner
    nh-outer so the sync-queue cols are needed only at matmul #5 of each
    step.  At each raw term's end the 8 psums are STAGED with bias folded
    in (stage := psum + bias, one DVE op per bank) so the next term's
    banks free at copy pace; C12/C21 then need no accumulator at all
    (their finals read stage + psum) and C22/C11 are created by 2-source
    stage - psum ops.  Terms 2-6 run TWO-PASS m-outer (pass 1 sweeps
    k0-3 for all four (nh0, nh1) pairs, pass 2 finishes k4-7 and evacs
    per pair) halving the B-prefix each term needs at its start.
  - dma_start CREATION order tracks consumption order (the runtime
    round-robins a small per-queue semaphore pool in program order).
    x half-tiles ride gpsimd in strict need order (k0-7 h0 for M3,
    k8-15 h1 for M4, k0-7 h1 for A2, k8-15 h0 for A5) - the 8-40us
    window (x + B3 + B4 + bias vs ~300GB/s of queues) is the bandwidth
    crunch, so late halves are deferred past it.  B k-tiles split
    [0:576] on scalar / [576:1024] on sync (1.125 vs 0.875 MB/term,
    matching the ~100/74 GB/s queue rates); the f32 bias streams in four
    256KB pieces right behind the B-chunks of the term whose stage needs
    them.  The B pool (bufs=3) gates each term's stream on the 3-ago
    term's readers - self-pacing.
  - C21/C12/C22 complete at terms 3/4/6 and stream out on gpsimd (idle
    after the x stream, one coalesced [128,1024] store per pair); C11
    completes at the final term and stores on the HWDGE queues (sync/
    scalar), with the last 512-col group split into 256+128+128-col
    pieces on separate PSUM tiles so the evac+store chain after the
    final matmul is short.
"""

import numpy as np
from contextlib import ExitStack

OUT = 2048
IN_BASE = 2048
N_EMBED = 64
IN_TOT = IN_BASE + 2 * N_EMBED  # 2176
BATCH = 8192
N_CORES = 8
B_CORE = BATCH // N_CORES       # 1024
P = 128
NW = 512                        # PSUM bank width (f32)
KT = 8                          # k-tiles per Strassen block (1024/128)
MT = 4                          # m-tiles per Strassen block (512/128)
NB = 1024                       # Strassen block edge (n and k)

_CACHED = {}


def _build_nc():
    import concourse.mybir as mybir
    import concourse.tile as tile
    from concourse import bacc

    f32 = mybir.dt.float32
    bf16 = mybir.dt.bfloat16

    nc = bacc.Bacc("TRN2", target_bir_lowering=False, debug=False,
                   num_devices=N_CORES)

    xt = nc.dram_tensor("xt", [IN_BASE, B_CORE], bf16, kind="ExternalInput")
    # 7 host-formed Strassen B operands, term order [B3,B4,B2,B5,B1,B6,B7],
    # each [1024 k, 1024 n], stacked -> [7*1024, 1024]
    wt = nc.dram_tensor("wt", [7 * NB, NB], bf16, kind="ExternalInput")
    bias = nc.dram_tensor("bias", [P, OUT], f32, kind="ExternalInput")
    out = nc.dram_tensor("out", [B_CORE, OUT], bf16, kind="ExternalOutput")

    xt_ap = xt.ap().rearrange("(k p) b -> p k b", p=P)    # [128, 16, 1024]
    wt_ap = wt.ap().rearrange("(ik p) n -> p ik n", p=P)  # [128, 56, 1024]

    # C block -> (row base, col base) in the [1024, 2048] output
    CBLK = {"C11": (0, 0), "C12": (0, NB), "C21": (512, 0), "C22": (512, NB)}

    # term list: (A source, [(C block, op), ...])
    #   A source: ("raw", k_base, half) slices xt_sb directly,
    #             ("sum", name) uses a DVE-formed A combination.
    # The two raw terms (M3, M4) are STAGED with bias folded in:
    #   stage0 := M3 + bias_hi,  stage1 := M4 + bias_lo  (one DVE op per
    # psum tile - frees the bank at copy pace).  Then C12 and C21 need no
    # accumulator at all (their finals read stage + psum directly), and
    # C22/C11 are *created* by 2-source stage - psum ops:
    #   C21 = stage1 + M2 (final)      C22 = stage0 - M2, += M1, final +M6
    #   C12 = stage0 + M5 (final)      C11 = stage1 - M5, += M1, final +M7
    TERMS = [
        (("raw", 0, 0), None),                                  # M3 -> stage0
        (("raw", 8, 1), None),                                  # M4 -> stage1
        (("sum", "A2"), [("C21", "final_s1"), ("C22", "create_s0")]),  # M2
        (("sum", "A5"), [("C12", "final_s0"), ("C11", "create_s1")]),  # M5
        (("sum", "A1"), [("C11", "add"), ("C22", "add")]),      # M1
        (("sum", "A6"), [("C22", "final")]),                    # M6
        (("sum", "A7"), [("C11", "final")]),                    # M7
    ]

    with tile.TileContext(nc) as tc:
        with ExitStack() as ctx:
            big_pool = ctx.enter_context(tc.tile_pool(name="big", bufs=1))
            b_pool = ctx.enter_context(tc.tile_pool(name="bops", bufs=3))
            asum_pool = ctx.enter_context(tc.tile_pool(name="asum", bufs=2))
            out_pool = ctx.enter_context(tc.tile_pool(name="out", bufs=4))
            psum_pool = ctx.enter_context(
                tc.tile_pool(name="psum", bufs=8, space="PSUM"))

            xt_sb = big_pool.tile([P, 16 * B_CORE], bf16, tag="xt")
            bias_t = big_pool.tile([P, OUT], f32, tag="bias")
            # f32 C accumulators - only C11/C22 need one (C12/C21
            # are formed directly from stage + psum at their final term)
            c_sb = {k: big_pool.tile([P, MT * NB], f32, tag=f"c_{k}",
                                     name=f"c_{k}")
                    for k in ("C11", "C22")}
            # DVE-formed A combinations live in a rolling 2-buffer pool
            # (each is used by exactly one term); filled by make_asum
            asum_sb = {}

            # PE p-state warmup: junk matmuls burn the HAM ramp while the
            # first input tiles are in flight.
            warm_pool = ctx.enter_context(tc.tile_pool(name="warm", bufs=1))
            wx = warm_pool.tile([P, P], bf16, tag="wx")
            ww = warm_pool.tile([P, 256], bf16, tag="ww")
            nc.vector.memset(wx[:], 0.0)
            nc.vector.memset(ww[:], 0.0)

            # DMA-ring warmup: absorb each ring's descriptor cold start
            for i, eng in enumerate((nc.sync, nc.scalar, nc.gpsimd)):
                scratch = warm_pool.tile([P, 128], bf16, tag=f"ringwarm{i}")
                eng.dma_start(out=scratch[:], in_=xt_ap[:, 15, 0:128])
            wps = psum_pool.tile([P, NW], f32, tag="ps", name="warm_ps")
            for _ in range(18):
                nc.tensor.matmul(wps[:, 0:256], lhsT=wx[:], rhs=ww[:],
                                 start=True, stop=True)

            # x chunk (g, h) = k-tile g, m-half h, as [128, 512] SBUF slice
            def xchunk(g, h):
                return xt_sb[:, g * B_CORE + h * NW:g * B_CORE + h * NW + NW]

            # B-term SBUF tiles (rolling pool of 3)
            bt = []

            # ---- DMA creation order == consumption order ----
            # x half-tiles on gpsimd in strict need order: k0-7 h0 (M3),
            # k8-15 h1 (M4), k0-7 h1 (A2, by ~42us), k8-15 h0 (A5, ~54us).
            # Deferring the late halves keeps the 8-40us crunch window
            # (x + B3 + B4 + bias vs ~300GB/s of queues) feasible.
            # B k-tiles split by need time within the k-step: scalar
            # [0:576] (matmuls 1-5), sync [576:1024] (matmuls 5-8) --
            # 1.125MB vs 0.875MB per term matches the ~100/74 GB/s queue
            # rates.  Bias rides in four 256KB pieces right behind the
            # B-chunks of the term whose stage needs them.
            NSC = 576
            bt.append(b_pool.tile([P, KT * NB], bf16, tag="bt",
                                  name="bt0"))
            for t in range(8):
                nc.gpsimd.dma_start(out=xchunk(t, 0),
                                    in_=xt_ap[:, t, 0:NW])
                nc.scalar.dma_start(
                    out=bt[0][:, t * NB:t * NB + NSC],
                    in_=wt_ap[:, t, 0:NSC])
                nc.sync.dma_start(
                    out=bt[0][:, t * NB + NSC:(t + 1) * NB],
                    in_=wt_ap[:, t, NSC:NB])
            nc.scalar.dma_start(out=bias_t[:, NB:NB + NW],
                                in_=bias.ap()[:, NB:NB + NW])
            nc.sync.dma_start(out=bias_t[:, NB + NW:],
                              in_=bias.ap()[:, NB + NW:])
            for t in range(8):
                nc.gpsimd.dma_start(out=xchunk(8 + t, 1),
                                    in_=xt_ap[:, 8 + t, NW:B_CORE])
            bt.append(b_pool.tile([P, KT * NB], bf16, tag="bt",
                                  name="bt1"))
            for t in range(KT):
                nc.scalar.dma_start(
                    out=bt[1][:, t * NB:t * NB + NSC],
                    in_=wt_ap[:, KT + t, 0:NSC])
                nc.sync.dma_start(
                    out=bt[1][:, t * NB + NSC:(t + 1) * NB],
                    in_=wt_ap[:, KT + t, NSC:NB])
            nc.scalar.dma_start(out=bias_t[:, 0:NW],
                                in_=bias.ap()[:, 0:NW])
            nc.sync.dma_start(out=bias_t[:, NW:NB],
                              in_=bias.ap()[:, NW:NB])
            for t in range(8):
                nc.gpsimd.dma_start(out=xchunk(t, 1),
                                    in_=xt_ap[:, t, NW:B_CORE])
            for t in range(8):
                nc.gpsimd.dma_start(out=xchunk(8 + t, 0),
                                    in_=xt_ap[:, 8 + t, 0:NW])
            for i in range(2, 7):
                bt.append(b_pool.tile([P, KT * NB], bf16, tag="bt",
                                      name=f"bt{i}"))
                # B2/B5's high cols ride gpsimd - idle and fast
                # (~150GB/s) once the x stream drains at ~35us, exactly
                # when these are needed; sync (74GB/s) can't deliver
                # them by t2/t3's pass-2 sweeps (measured 3.6us stall)
                hi_eng = nc.gpsimd if i in (2, 3) else nc.sync
                for t in range(KT):
                    nc.scalar.dma_start(
                        out=bt[i][:, t * NB:t * NB + NSC],
                        in_=wt_ap[:, i * KT + t, 0:NSC])
                    hi_eng.dma_start(
                        out=bt[i][:, t * NB + NSC:(t + 1) * NB],
                        in_=wt_ap[:, i * KT + t, NSC:NB])

            # ---- A combinations (DVE, bf16; scheduler slots them by deps)
            def aslice(name, t):
                return asum_sb[name][:, t * NW:(t + 1) * NW]

            # Wait stamps (ms) for the first two A-sums: the scheduler's
            # optimistic DMA estimate would otherwise order them BEFORE
            # the M3/M4 stage evacs on DVE, head-of-line blocking the
            # next term's PSUM banks behind a late x transfer (measured
            # 11us).  A1/A6/A7 read fully-resident x, so their estimated
            # readiness is accurate - stamping them too late was measured
            # to stretch the end-game instead.
            ASUM_WAIT = {"A2": 0.034, "A5": 0.037}

            def make_asum(name):
                defs = {
                    # A1 = A11 + A22, A2 = A21 + A22, A5 = A11 + A12,
                    # A6 = A21 - A11, A7 = A12 - A22
                    "A1": (False, lambda t: (xchunk(t, 0), xchunk(8 + t, 1))),
                    "A2": (False, lambda t: (xchunk(t, 1), xchunk(8 + t, 1))),
                    "A5": (False, lambda t: (xchunk(t, 0), xchunk(8 + t, 0))),
                    "A6": (True, lambda t: (xchunk(t, 1), xchunk(t, 0))),
                    "A7": (True, lambda t: (xchunk(8 + t, 0),
                                            xchunk(8 + t, 1))),
                }
                sub, operands = defs[name]
                asum_sb[name] = asum_pool.tile([P, KT * NW], bf16, tag="as",
                                               name=f"as_{name}")
                op = nc.vector.tensor_sub if sub else nc.vector.tensor_add
                with tc.tile_wait_until(ASUM_WAIT.get(name, 0),
                                        enable=name in ASUM_WAIT):
                    for t in range(KT):
                        a, b = operands(t)
                        op(aslice(name, t), a, b)

            # lhsT accessor for term i, k-tile k, m-tile m -> [128, 128]
            def lhsT(asrc, k, m):
                if asrc[0] == "raw":
                    _, kb, h = asrc
                    base = (kb + k) * B_CORE + h * NW
                    return xt_sb[:, base + m * P:base + (m + 1) * P]
                name = asrc[1]
                return asum_sb[name][:, k * NW + m * P:k * NW + (m + 1) * P]

            def rhs(i, k, nh):
                return bt[i][:, k * NB + nh * NW:k * NB + (nh + 1) * NW]

            def cslice(cname, m, nh):
                return c_sb[cname][:, m * NB + nh * NW:
                                   m * NB + (nh + 1) * NW]

            def bslice(cname, nh):
                cb = CBLK[cname][1]
                return bias_t[:, cb + nh * NW:cb + (nh + 1) * NW]

            # evac one psum tile per this term's C ops (all on DVE -
            # GPSIMD cannot access PSUM).
            #   final_sX  out := stageX + psum   (bf16, no accumulator)
            #   create_sX C := stageX - psum     (first write of C)
            #   add       C += psum
            #   final     out := C + psum
            def evac(ops, psum, m, nh, ot_pair=None):
                g = nh * MT + m
                for cname, op in ops:
                    if op == "final_s0" or op == "final_s1":
                        st = stage[0 if op[-1] == "0" else 1]
                        nc.vector.tensor_add(
                            ot_pair[:, nh * NW:(nh + 1) * NW],
                            st[:, g * NW:(g + 1) * NW], psum[:])
                        continue
                    csl = cslice(cname, m, nh)
                    if op == "create_s0" or op == "create_s1":
                        st = stage[0 if op[-1] == "0" else 1]
                        nc.vector.tensor_sub(
                            csl, st[:, g * NW:(g + 1) * NW], psum[:])
                    elif op == "add":
                        nc.vector.tensor_add(csl, csl, psum[:])
                    else:  # final
                        nc.vector.tensor_add(
                            ot_pair[:, nh * NW:(nh + 1) * NW], csl, psum[:])

            # ---- terms 0,1 (M3, M4): k-outer, paced by the live streams.
            # nh-outer step order: the nh1 cols ride the slower queues and
            # are first needed at matmul #5 of the step.  At term end each
            # psum is staged with bias folded in (ONE DVE op per bank:
            # stageX := psum + bias) so the next term's banks free at copy
            # pace - no separate C-init pass exists at all.
            stage = [big_pool.tile([P, 8 * NW], f32, tag=f"stage{j}",
                                   name=f"stage{j}") for j in range(2)]

            for i in (0, 1):
                asrc, _ = TERMS[i]
                bias_cb = NB if i == 0 else 0   # M3 feeds C12/C22 (hi cols)
                psums = [psum_pool.tile([P, NW], f32, tag="ps",
                                        name=f"ps_t{i}_g{g}")
                         for g in range(8)]
                for k in range(KT):
                    for nh in range(2):
                        for m in range(MT):
                            nc.tensor.matmul(
                                psums[nh * MT + m][:],
                                lhsT=lhsT(asrc, k, m), rhs=rhs(i, k, nh),
                                start=(k == 0), stop=(k == KT - 1))
                for g in range(8):
                    nh = g // MT
                    nc.vector.tensor_add(
                        stage[i][:, g * NW:(g + 1) * NW], psums[g][:],
                        bias_t[:, bias_cb + nh * NW:bias_cb + (nh + 1) * NW])
                if i == 0:
                    make_asum("A2")
                    make_asum("A5")

            # ---- terms 2..6: TWO-PASS m-outer (pass 1 sweeps k0-3 for
            # all pairs, pass 2 finishes k4-7 then evacs) - halves the
            # B-prefix a term needs at its start, so the streamed B keeps
            # up; evacs still trickle per pair and pair stores coalesce
            # into one [128, 1024] tile (one SWDGE issue per pair)
            NEXT_ASUM = {2: "A1", 3: "A6", 4: "A7"}
            for i in range(2, 7):
                asrc, ops = TERMS[i]
                final_c = next((c for c, op in ops
                                if op.startswith("final")), None)
                last = i == 6
                m_full = MT - 1 if last else MT
                prs = []
                for m in range(m_full):
                    psa = psum_pool.tile([P, NW], f32, tag="ps",
                                         name=f"ps_t{i}_m{m}n0")
                    psb = psum_pool.tile([P, NW], f32, tag="ps",
                                         name=f"ps_t{i}_m{m}n1")
                    prs.append((psa, psb))
                    for k in range(KT // 2):
                        nc.tensor.matmul(
                            psa[:], lhsT=lhsT(asrc, k, m), rhs=rhs(i, k, 0),
                            start=(k == 0), stop=False)
                        nc.tensor.matmul(
                            psb[:], lhsT=lhsT(asrc, k, m), rhs=rhs(i, k, 1),
                            start=(k == 0), stop=False)
                for m in range(m_full):
                    psa, psb = prs[m]
                    for k in range(KT // 2, KT):
                        nc.tensor.matmul(
                            psa[:], lhsT=lhsT(asrc, k, m), rhs=rhs(i, k, 0),
                            start=False, stop=(k == KT - 1))
                        nc.tensor.matmul(
                            psb[:], lhsT=lhsT(asrc, k, m), rhs=rhs(i, k, 1),
                            start=False, stop=(k == KT - 1))
                    ot_pair = None
                    if final_c is not None:
                        ot_pair = out_pool.tile([P, NB], bf16, tag="otp")
                    evac(ops, psa, m, 0, ot_pair)
                    evac(ops, psb, m, 1, ot_pair)
                    if final_c is not None:
                        rb, cb = CBLK[final_c]
                        st = (nc.sync if m % 2 == 0 else nc.scalar) \
                            if last else nc.gpsimd
                        st.dma_start(
                            out=out.ap()[rb + m * P:rb + (m + 1) * P,
                                         cb:cb + NB],
                            in_=ot_pair[:])
                if i in NEXT_ASUM:
                    make_asum(NEXT_ASUM[i])
                if not last:
                    continue
                # final m-tile of the final term (C11 rows 384:512):
                # nh0 as one group, nh1 split 256+128+128 on separate PSUM
                # tiles so the post-final-matmul evac+store chain is short
                m = MT - 1
                ps = psum_pool.tile([P, NW], f32, tag="ps",
                                    name=f"ps_t{i}_m{m}n0")
                for k in range(KT):
                    nc.tensor.matmul(
                        ps[:], lhsT=lhsT(asrc, k, m), rhs=rhs(i, k, 0),
                        start=(k == 0), stop=(k == KT - 1))
                rb, cb = CBLK["C11"]
                otn0 = out_pool.tile([P, NW], bf16, tag="otn0")
                nc.vector.tensor_add(otn0[:], cslice("C11", m, 0), ps[:])
                nc.sync.dma_start(
                    out=out.ap()[rb + m * P:rb + (m + 1) * P, cb:cb + NW],
                    in_=otn0[:])
                pieces = [(0, NW // 2), (NW // 2, NW // 4),
                          (3 * NW // 4, NW // 4)]
                for h, (off, w) in enumerate(pieces):
                    c0 = NW + off  # within C11's 1024 cols (nh1 window)
                    psh = psum_pool.tile([P, NW], f32, tag="ps",
                                         name=f"ps_t{i}_m{m}h{h}")
                    for k in range(KT):
                        nc.tensor.matmul(
                            psh[:, 0:w],
                            lhsT=lhsT(asrc, k, m),
                            rhs=bt[i][:, k * NB + c0:k * NB + c0 + w],
                            start=(k == 0), stop=(k == KT - 1))
                    ot = out_pool.tile([P, w], bf16, tag=f"oth{w}")
                    nc.vector.tensor_add(
                        ot[:], c_sb["C11"][:, m * NB + c0:m * NB + c0 + w],
                        psh[:, 0:w])
                    st = (nc.scalar, nc.sync, nc.scalar)[h]
                    st.dma_start(
                        out=out.ap()[rb + m * P:rb + (m + 1) * P,
                                     cb + c0:cb + c0 + w],
                        in_=ot[:])

    nc.compile()
    return nc


def _host_prep(inputs):
    import ml_dtypes

    x = np.asarray(inputs["x"], dtype=np.float32)
    wv = np.asarray(inputs["weight_vals"], dtype=np.float32)
    wr = np.asarray(inputs["weight_rows"]).astype(np.int64)
    wc = np.asarray(inputs["weight_cols"]).astype(np.int64)
    bv = np.asarray(inputs["bias_vals"], dtype=np.float32)
    bi = np.asarray(inputs["bias_idx"]).astype(np.int64)
    e0v = np.asarray(inputs["embed0_vals"], dtype=np.float32)
    e0p = np.asarray(inputs["embed0_parents"]).astype(np.int64)
    e1v = np.asarray(inputs["embed1_vals"], dtype=np.float32)
    e1p = np.asarray(inputs["embed1_parents"]).astype(np.int64)

    # dense W^T [IN_TOT, OUT] (coalesce: duplicates sum)
    wt_full = np.bincount(wc * OUT + wr, weights=wv,
                          minlength=IN_TOT * OUT).reshape(IN_TOT, OUT)

    # resolve embed parent chains to (row-in-x, multiplier), then fold the
    # expanded-feature rows of W^T into their parent rows
    q = np.empty(2 * N_EMBED, dtype=np.int64)
    a = np.empty(2 * N_EMBED, dtype=np.float64)
    q[:N_EMBED] = e0p
    a[:N_EMBED] = e0v
    for j in range(N_EMBED):
        p = int(e1p[j])
        if p < IN_BASE:
            q[N_EMBED + j] = p
            a[N_EMBED + j] = e1v[j]
        else:
            t = p - IN_BASE
            q[N_EMBED + j] = e0p[t]
            a[N_EMBED + j] = e1v[j] * e0v[t]
    wt_eff = wt_full[:IN_BASE]
    np.add.at(wt_eff, q, a[:, None] * wt_full[IN_BASE:])
    wt_eff = wt_eff.astype(np.float32)  # [2048 k, 2048 n]

    # host-side Strassen B combinations, stacked in TERM order
    B11 = wt_eff[:NB, :NB]
    B12 = wt_eff[:NB, NB:]
    B21 = wt_eff[NB:, :NB]
    B22 = wt_eff[NB:, NB:]
    bs = np.concatenate([
        B12 - B22,   # B3
        B21 - B11,   # B4
        B11,         # B2
        B22,         # B5
        B11 + B22,   # B1
        B11 + B12,   # B6
        B21 + B22,   # B7
    ], axis=0)
    wt_bf = np.ascontiguousarray(bs.astype(ml_dtypes.bfloat16))

    b = np.bincount(bi, weights=bv, minlength=OUT).astype(np.float32)
    bias_row = np.ascontiguousarray(np.broadcast_to(b[None, :], (128, OUT)))

    x_bf = x.astype(ml_dtypes.bfloat16)
    xts = [np.ascontiguousarray(x_bf[i * B_CORE:(i + 1) * B_CORE].T)
           for i in range(N_CORES)]
    return xts, wt_bf, bias_row


def kernel(**inputs) -> np.ndarray:
    import time
    from concourse.bass_utils import run_bass_kernel_spmd

    if "nc" not in _CACHED:
        _CACHED["nc"] = _build_nc()
    nc = _CACHED["nc"]

    xts, wt_bf, bias_row = _host_prep(inputs)
    in_maps = [dict(xt=xts[i], wt=wt_bf, bias=bias_row)
               for i in range(N_CORES)]
    res = None
    last_exc = None
    for attempt in range(3):
        try:
            res = run_bass_kernel_spmd(nc, in_maps,
                                       core_ids=list(range(N_CORES)))
            break
        except Exception as e:  # transient device/runtime hiccups
            last_exc = e
            time.sleep(2.0)
    if res is None:
        raise last_exc
    out = np.concatenate([res.results[i]["out"] for i in range(N_CORES)],
                         axis=0)
    return np.ascontiguousarray(out.astype(np.float32))


# revision 24
# speedup vs baseline: 1.0198x; 1.0198x over previous
"""Trainium2 Bass kernel for nn_ExpandingLinear.

Reference computation:
    x_exp = concat([x, x[:, p0] * v0, x_exp1[:, p1] * v1], axis=1)   # [B, 2176]
    W     = scatter_add(weight_vals at [weight_rows, weight_cols])    # [2048, 2176]
    b     = scatter_add(bias_vals at bias_idx)                        # [2048]
    out   = x_exp @ W.T + b                                           # [B, 2048]

Every expanded feature c is a_c * x[:, q_c] for a resolvable (q_c, a_c)
(parent chains only reference earlier features), so the embed columns fold
into the base weight on the host:
    W_eff[o, q_c] += a_c * W[o, 2048 + c]      ->  out = x @ W_eff.T + b
which reduces the device work to a dense [1024, 2048] @ [2048, 2048]
matmul + bias per core (data-parallel batch shard, 8 cores).

The classic dense schedule is issue-rate-bound on the PE: 512 matmuls of
[128,128]@[128,512] at 1 col/cycle = 109.2us, measured gapless.  To go
below that floor this kernel uses ONE-LEVEL STRASSEN on the per-core
matmul C[1024,2048] = x[1024,2048] @ Wt[2048,2048]:

    A11..A22 = 2x2 blocks of x   ([512,1024] each)
    B11..B22 = 2x2 blocks of Wt  ([1024,1024] each)
    M1 = (A11+A22)(B11+B22)  M2 = (A21+A22)B11   M3 = A11(B12-B22)
    M4 = A22(B21-B11)        M5 = (A11+A12)B22   M6 = (A21-A11)(B11+B12)
    M7 = (A12-A22)(B21+B22)
    C11 = M1+M4-M5+M7   C12 = M3+M5   C21 = M2+M4   C22 = M1-M2+M3+M6

7 products of [512,1024]@[1024,1024] = 7 * 64 matmuls = 95.6us PE stream
(12.5% less).  The 7 B-combinations are formed ON THE HOST (free) and
streamed; the 5 A-combinations are formed on the vector engine from the
resident x (bf16 adds, hidden under the PE stream).  M-terms accumulate in
PSUM (8 banks = the 8 concurrent groups of a term); GPSIMD cannot read
PSUM, so every psum-consuming op runs on DVE.

Numerics: one Strassen level grows bf16 error only slightly (measured
5.4e-3 vs 3.4e-3 classic, gate 2e-2).  PSUM accumulates fp32; C combines
are fp32.

Device schedule (per core), ~127.4us HW exec (classic dense: 128.3us):
  - PE p-state warmup (junk matmuls) burns the HAM 0.65->2.4GHz ramp while
    the first input tiles are in flight; 128-partition ring-warmup reads
    absorb each DMA queue's per-engine descriptor cold start.
  - term order M3, M4, M2, M5, M1, M6, M7.  M3 (lhsT = raw A11) and M4
    (raw A22) run k-outer paced by the live x/B streams, step-inner
    nh-outer so the sync-queue cols are needed only at matmul #5 of each
    step.  At each raw term's end the 8 psums are STAGED with bias folded
    in (stage := psum + bias, one DVE op per bank) so the next term's
    banks free at copy pace; C12/C21 then need no accumulator at all
    (their finals read stage + psum) and C22/C11 are created by 2-source
    stage - psum ops.  Terms 2-6 run TWO-PASS m-outer (pass 1 sweeps
    k0-3 for all four (nh0, nh1) pairs, pass 2 finishes k4-7 and evacs
    per pair) halving the B-prefix each term needs at its start.
  - dma_start CREATION order tracks consumption order (the runtime
    round-robins a small per-queue semaphore pool in program order).
    x half-tiles ride gpsimd in strict need order (k0-7 h0 for M3,
    k8-15 h1 for M4, k0-7 h1 for A2, k8-15 h0 for A5) - the 8-40us
    window (x + B3 + B4 + bias vs ~300GB/s of queues) is the bandwidth
    crunch, so late halves are deferred past it.  B k-tiles split
    [0:576] on scalar / [576:1024] on sync (1.125 vs 0.875 MB/term,
    matching the ~100/74 GB/s queue rates); the f32 bias streams in four
    256KB pieces right behind the B-chunks of the term whose stage needs
    them.  The B pool (bufs=3) gates each term's stream on the 3-ago
    term's readers - self-pacing.
  - C21/C12/C22 complete at terms 3/4/6 and stream out on gpsimd (idle
    after the x stream, one coalesced [128,1024] store per pair); C11
    completes at the final term and stores on the HWDGE queues (sync/
    scalar), with the last 512-col group split into 256+128+128-col
    pieces on separate PSUM tiles so the evac+store chain after the
    final matmul is short.
"""

import numpy as np
from contextlib import ExitStack

OUT = 2048
IN_BASE = 2048
N_EMBED = 64
IN_TOT = IN_BASE + 2 * N_EMBED  # 2176
BATCH = 8192
N_CORES = 8
B_CORE = BATCH // N_CORES       # 1024
P = 128
NW = 512                        # PSUM bank width (f32)
KT = 8                          # k-tiles per Strassen block (1024/128)
MT = 4                          # m-tiles per Strassen block (512/128)
NB = 1024                       # Strassen block edge (n and k)

_CACHED = {}


def _build_nc():
    import concourse.mybir as mybir
    import concourse.tile as tile
    from concourse import bacc

    f32 = mybir.dt.float32
    bf16 = mybir.dt.bfloat16

    nc = bacc.Bacc("TRN2", target_bir_lowering=False, debug=False,
                   num_devices=N_CORES)

    xt = nc.dram_tensor("xt", [IN_BASE, B_CORE], bf16, kind="ExternalInput")
    # 7 host-formed Strassen B operands, term order [B3,B4,B2,B5,B1,B6,B7],
    # each [1024 k, 1024 n], stacked -> [7*1024, 1024]
    wt = nc.dram_tensor("wt", [7 * NB, NB], bf16, kind="ExternalInput")
    bias = nc.dram_tensor("bias", [P, OUT], f32, kind="ExternalInput")
    out = nc.dram_tensor("out", [B_CORE, OUT], bf16, kind="ExternalOutput")

    xt_ap = xt.ap().rearrange("(k p) b -> p k b", p=P)    # [128, 16, 1024]
    wt_ap = wt.ap().rearrange("(ik p) n -> p ik n", p=P)  # [128, 56, 1024]

    # C block -> (row base, col base) in the [1024, 2048] output
    CBLK = {"C11": (0, 0), "C12": (0, NB), "C21": (512, 0), "C22": (512, NB)}

    # term list: (A source, [(C block, op), ...])
    #   A source: ("raw", k_base, half) slices xt_sb directly,
    #             ("sum", name) uses a DVE-formed A combination.
    # The two raw terms (M3, M4) are STAGED with bias folded in:
    #   stage0 := M3 + bias_hi,  stage1 := M4 + bias_lo  (one DVE op per
    # psum tile - frees the bank at copy pace).  Then C12 and C21 need no
    # accumulator at all (their finals read stage + psum directly), and
    # C22/C11 are *created* by 2-source stage - psum ops:
    #   C21 = stage1 + M2 (final)      C22 = stage0 - M2, += M1, final +M6
    #   C12 = stage0 + M5 (final)      C11 = stage1 - M5, += M1, final +M7
    TERMS = [
        (("raw", 0, 0), None),                                  # M3 -> stage0
        (("raw", 8, 1), None),                                  # M4 -> stage1
        (("sum", "A2"), [("C21", "final_s1"), ("C22", "create_s0")]),  # M2
        (("sum", "A5"), [("C12", "final_s0"), ("C11", "create_s1")]),  # M5
        (("sum", "A1"), [("C11", "add"), ("C22", "add")]),      # M1
        (("sum", "A6"), [("C22", "final")]),                    # M6
        (("sum", "A7"), [("C11", "final")]),                    # M7
    ]

    with tile.TileContext(nc) as tc:
        with ExitStack() as ctx:
            big_pool = ctx.enter_context(tc.tile_pool(name="big", bufs=1))
            b_pool = ctx.enter_context(tc.tile_pool(name="bops", bufs=3))
            asum_pool = ctx.enter_context(tc.tile_pool(name="asum", bufs=2))
            out_pool = ctx.enter_context(tc.tile_pool(name="out", bufs=4))
            psum_pool = ctx.enter_context(
                tc.tile_pool(name="psum", bufs=8, space="PSUM"))

            xt_sb = big_pool.tile([P, 16 * B_CORE], bf16, tag="xt")
            bias_t = big_pool.tile([P, OUT], f32, tag="bias")
            # f32 C accumulators - only C11/C22 need one (C12/C21
            # are formed directly from stage + psum at their final term)
            c_sb = {k: big_pool.tile([P, MT * NB], f32, tag=f"c_{k}",
                                     name=f"c_{k}")
                    for k in ("C11", "C22")}
            # DVE-formed A combinations live in a rolling 2-buffer pool
            # (each is used by exactly one term); filled by make_asum
            asum_sb = {}

            # PE p-state warmup: junk matmuls burn the HAM ramp while the
            # first input tiles are in flight.
            warm_pool = ctx.enter_context(tc.tile_pool(name="warm", bufs=1))
            wx = warm_pool.tile([P, P], bf16, tag="wx")
            ww = warm_pool.tile([P, 256], bf16, tag="ww")
            nc.vector.memset(wx[:], 0.0)
            nc.vector.memset(ww[:], 0.0)

            # DMA-ring warmup: absorb each ring's descriptor cold start
            for i, eng in enumerate((nc.sync, nc.scalar, nc.gpsimd)):
                scratch = warm_pool.tile([P, 128], bf16, tag=f"ringwarm{i}")
                eng.dma_start(out=scratch[:], in_=xt_ap[:, 15, 0:128])
            wps = psum_pool.tile([P, NW], f32, tag="ps", name="warm_ps")
            for _ in range(18):
                nc.tensor.matmul(wps[:, 0:256], lhsT=wx[:], rhs=ww[:],
                                 start=True, stop=True)

            # x chunk (g, h) = k-tile g, m-half h, as [128, 512] SBUF slice
            def xchunk(g, h):
                return xt_sb[:, g * B_CORE + h * NW:g * B_CORE + h * NW + NW]

            # B-term SBUF tiles (rolling pool of 3)
            bt = []

            # ---- DMA creation order == consumption order ----
            # x half-tiles on gpsimd in strict need order: k0-7 h0 (M3),
            # k8-15 h1 (M4), k0-7 h1 (A2, by ~42us), k8-15 h0 (A5, ~54us).
            # Deferring the late halves keeps the 8-40us crunch window
            # (x + B3 + B4 + bias vs ~300GB/s of queues) feasible.
            # B k-tiles split by need time within the k-step: scalar
            # [0:576] (matmuls 1-5), sync [576:1024] (matmuls 5-8) --
            # 1.125MB vs 0.875MB per term matches the ~100/74 GB/s queue
            # rates.  Bias rides in four 256KB pieces right behind the
            # B-chunks of the term whose stage needs them.
            NSC = 576
            bt.append(b_pool.tile([P, KT * NB], bf16, tag="bt",
                                  name="bt0"))
            for t in range(8):
                nc.gpsimd.dma_start(out=xchunk(t, 0),
                                    in_=xt_ap[:, t, 0:NW])
                nc.scalar.dma_start(
                    out=bt[0][:, t * NB:t * NB + NSC],
                    in_=wt_ap[:, t, 0:NSC])
                nc.sync.dma_start(
                    out=bt[0][:, t * NB + NSC:(t + 1) * NB],
                    in_=wt_ap[:, t, NSC:NB])
            nc.scalar.dma_start(out=bias_t[:, NB:NB + NW],
                                in_=bias.ap()[:, NB:NB + NW])
            nc.sync.dma_start(out=bias_t[:, NB + NW:],
                              in_=bias.ap()[:, NB + NW:])
            for t in range(8):
                nc.gpsimd.dma_start(out=xchunk(8 + t, 1),
                                    in_=xt_ap[:, 8 + t, NW:B_CORE])
            bt.append(b_pool.tile([P, KT * NB], bf16, tag="bt",
                                  name="bt1"))
            for t in range(KT):
                nc.scalar.dma_start(
                    out=bt[1][:, t * NB:t * NB + NSC],
                    in_=wt_ap[:, KT + t, 0:NSC])
                nc.sync.dma_start(
                    out=bt[1][:, t * NB + NSC:(t + 1) * NB],
                    in_=wt_ap[:, KT + t, NSC:NB])
            nc.scalar.dma_start(out=bias_t[:, 0:NW],
                                in_=bias.ap()[:, 0:NW])
            nc.sync.dma_start(out=bias_t[:, NW:NB],
                              in_=bias.ap()[:, NW:NB])
            for t in range(8):
                nc.gpsimd.dma_start(out=xchunk(t, 1),
                                    in_=xt_ap[:, t, NW:B_CORE])
            for t in range(8):
                nc.gpsimd.dma_start(out=xchunk(8 + t, 0),
                                    in_=xt_ap[:, 8 + t, 0:NW])
            for i in range(2, 7):
                bt.append(b_pool.tile([P, KT * NB], bf16, tag="bt",
                                      name=f"bt{i}"))
                for t in range(KT):
                    nc.scalar.dma_start(
                        out=bt[i][:, t * NB:t * NB + NSC],
                        in_=wt_ap[:, i * KT + t, 0:NSC])
                    nc.sync.dma_start(
                        out=bt[i][:, t * NB + NSC:(t + 1) * NB],
                        in_=wt_ap[:, i * KT + t, NSC:NB])

            # ---- A combinations (DVE, bf16; scheduler slots them by deps)
            def aslice(name, t):
                return asum_sb[name][:, t * NW:(t + 1) * NW]

            def make_asum(name):
                defs = {
                    # A1 = A11 + A22, A2 = A21 + A22, A5 = A11 + A12,
                    # A6 = A21 - A11, A7 = A12 - A22
                    "A1": (False, lambda t: (xchunk(t, 0), xchunk(8 + t, 1))),
                    "A2": (False, lambda t: (xchunk(t, 1), xchunk(8 + t, 1))),
                    "A5": (False, lambda t: (xchunk(t, 0), xchunk(8 + t, 0))),
                    "A6": (True, lambda t: (xchunk(t, 1), xchunk(t, 0))),
                    "A7": (True, lambda t: (xchunk(8 + t, 0),
                                            xchunk(8 + t, 1))),
                }
                sub, operands = defs[name]
                asum_sb[name] = asum_pool.tile([P, KT * NW], bf16, tag="as",
                                               name=f"as_{name}")
                op = nc.vector.tensor_sub if sub else nc.vector.tensor_add
                for t in range(KT):
                    a, b = operands(t)
                    op(aslice(name, t), a, b)

            # lhsT accessor for term i, k-tile k, m-tile m -> [128, 128]
            def lhsT(asrc, k, m):
                if asrc[0] == "raw":
                    _, kb, h = asrc
                    base = (kb + k) * B_CORE + h * NW
                    return xt_sb[:, base + m * P:base + (m + 1) * P]
                name = asrc[1]
                return asum_sb[name][:, k * NW + m * P:k * NW + (m + 1) * P]

            def rhs(i, k, nh):
                return bt[i][:, k * NB + nh * NW:k * NB + (nh + 1) * NW]

            def cslice(cname, m, nh):
                return c_sb[cname][:, m * NB + nh * NW:
                                   m * NB + (nh + 1) * NW]

            def bslice(cname, nh):
                cb = CBLK[cname][1]
                return bias_t[:, cb + nh * NW:cb + (nh + 1) * NW]

            # evac one psum tile per this term's C ops (all on DVE -
            # GPSIMD cannot access PSUM).
            #   final_sX  out := stageX + psum   (bf16, no accumulator)
            #   create_sX C := stageX - psum     (first write of C)
            #   add       C += psum
            #   final     out := C + psum
            def evac(ops, psum, m, nh, ot_pair=None):
                g = nh * MT + m
                for cname, op in ops:
                    if op == "final_s0" or op == "final_s1":
                        st = stage[0 if op[-1] == "0" else 1]
                        nc.vector.tensor_add(
                            ot_pair[:, nh * NW:(nh + 1) * NW],
                            st[:, g * NW:(g + 1) * NW], psum[:])
                        continue
                    csl = cslice(cname, m, nh)
                    if op == "create_s0" or op == "create_s1":
                        st = stage[0 if op[-1] == "0" else 1]
                        nc.vector.tensor_sub(
                            csl, st[:, g * NW:(g + 1) * NW], psum[:])
                    elif op == "add":
                        nc.vector.tensor_add(csl, csl, psum[:])
                    else:  # final
                        nc.vector.tensor_add(
                            ot_pair[:, nh * NW:(nh + 1) * NW], csl, psum[:])

            # ---- terms 0,1 (M3, M4): k-outer, paced by the live streams.
            # nh-outer step order: the nh1 cols ride the slower queues and
            # are first needed at matmul #5 of the step.  At term end each
            # psum is staged with bias folded in (ONE DVE op per bank:
            # stageX := psum + bias) so the next term's banks free at copy
            # pace - no separate C-init pass exists at all.
            stage = [big_pool.tile([P, 8 * NW], f32, tag=f"stage{j}",
                                   name=f"stage{j}") for j in range(2)]

            for i in (0, 1):
                asrc, _ = TERMS[i]
                bias_cb = NB if i == 0 else 0   # M3 feeds C12/C22 (hi cols)
                psums = [psum_pool.tile([P, NW], f32, tag="ps",
                                        name=f"ps_t{i}_g{g}")
                         for g in range(8)]
                for k in range(KT):
                    for nh in range(2):
                        for m in range(MT):
                            nc.tensor.matmul(
                                psums[nh * MT + m][:],
                                lhsT=lhsT(asrc, k, m), rhs=rhs(i, k, nh),
                                start=(k == 0), stop=(k == KT - 1))
                for g in range(8):
                    nh = g // MT
                    nc.vector.tensor_add(
                        stage[i][:, g * NW:(g + 1) * NW], psums[g][:],
                        bias_t[:, bias_cb + nh * NW:bias_cb + (nh + 1) * NW])
                if i == 0:
                    make_asum("A2")
                    make_asum("A5")

            # ---- terms 2..6: TWO-PASS m-outer (pass 1 sweeps k0-3 for
            # all pairs, pass 2 finishes k4-7 then evacs) - halves the
            # B-prefix a term needs at its start, so the streamed B keeps
            # up; evacs still trickle per pair and pair stores coalesce
            # into one [128, 1024] tile (one SWDGE issue per pair)
            NEXT_ASUM = {2: "A1", 3: "A6", 4: "A7"}
            for i in range(2, 7):
                asrc, ops = TERMS[i]
                final_c = next((c for c, op in ops
                                if op.startswith("final")), None)
                last = i == 6
                m_full = MT - 1 if last else MT
                prs = []
                for m in range(m_full):
                    psa = psum_pool.tile([P, NW], f32, tag="ps",
                                         name=f"ps_t{i}_m{m}n0")
                    psb = psum_pool.tile([P, NW], f32, tag="ps",
                                         name=f"ps_t{i}_m{m}n1")
                    prs.append((psa, psb))
                    for k in range(KT // 2):
                        nc.tensor.matmul(
                            psa[:], lhsT=lhsT(asrc, k, m), rhs=rhs(i, k, 0),
                            start=(k == 0), stop=False)
                        nc.tensor.matmul(
                            psb[:], lhsT=lhsT(asrc, k, m), rhs=rhs(i, k, 1),
                            start=(k == 0), stop=False)
                for m in range(m_full):
                    psa, psb = prs[m]
                    for k in range(KT // 2, KT):
                        nc.tensor.matmul(
                            psa[:], lhsT=lhsT(asrc, k, m), rhs=rhs(i, k, 0),
                            start=False, stop=(k == KT - 1))
                        nc.tensor.matmul(
                            psb[:], lhsT=lhsT(asrc, k, m), rhs=rhs(i, k, 1),
                            start=False, stop=(k == KT - 1))
                    ot_pair = None
                    if final_c is not None:
                        ot_pair = out_pool.tile([P, NB], bf16, tag="otp")
                    evac(ops, psa, m, 0, ot_pair)
                    evac(ops, psb, m, 1, ot_pair)
                    if final_c is not None:
                        rb, cb = CBLK[final_c]
                        st = (nc.sync if m % 2 == 0 else nc.scalar) \
                            if last else nc.gpsimd
                        st.dma_start(
                            out=out.ap()[rb + m * P:rb + (m + 1) * P,
                                         cb:cb + NB],
                            in_=ot_pair[:])
                if i in NEXT_ASUM:
                    make_asum(NEXT_ASUM[i])
                if not last:
                    continue
                # final m-tile of the final term (C11 rows 384:512):
                # nh0 as one group, nh1 split 256+128+128 on separate PSUM
                # tiles so the post-final-matmul evac+store chain is short
                m = MT - 1
                ps = psum_pool.tile([P, NW], f32, tag="ps",
                                    name=f"ps_t{i}_m{m}n0")
                for k in range(KT):
                    nc.tensor.matmul(
                        ps[:], lhsT=lhsT(asrc, k, m), rhs=rhs(i, k, 0),
                        start=(k == 0), stop=(k == KT - 1))
                rb, cb = CBLK["C11"]
                otn0 = out_pool.tile([P, NW], bf16, tag="otn0")
                nc.vector.tensor_add(otn0[:], cslice("C11", m, 0), ps[:])
                nc.sync.dma_start(
                    out=out.ap()[rb + m * P:rb + (m + 1) * P, cb:cb + NW],
                    in_=otn0[:])
                pieces = [(0, NW // 2), (NW // 2, NW // 4),
                          (3 * NW // 4, NW // 4)]
                for h, (off, w) in enumerate(pieces):
                    c0 = NW + off  # within C11's 1024 cols (nh1 window)
                    psh = psum_pool.tile([P, NW], f32, tag="ps",
                                         name=f"ps_t{i}_m{m}h{h}")
                    for k in range(KT):
                        nc.tensor.matmul(
                            psh[:, 0:w],
                            lhsT=lhsT(asrc, k, m),
                            rhs=bt[i][:, k * NB + c0:k * NB + c0 + w],
                            start=(k == 0), stop=(k == KT - 1))
                    ot = out_pool.tile([P, w], bf16, tag=f"oth{w}")
                    nc.vector.tensor_add(
                        ot[:], c_sb["C11"][:, m * NB + c0:m * NB + c0 + w],
                        psh[:, 0:w])
                    st = (nc.scalar, nc.sync, nc.scalar)[h]
                    st.dma_start(
                        out=out.ap()[rb + m * P:rb + (m + 1) * P,
                                     cb + c0:cb + c0 + w],
                        in_=ot[:])

    nc.compile()
    return nc


def _host_prep(inputs):
    import ml_dtypes

    x = np.asarray(inputs["x"], dtype=np.float32)
    wv = np.asarray(inputs["weight_vals"], dtype=np.float32)
    wr = np.asarray(inputs["weight_rows"]).astype(np.int64)
    wc = np.asarray(inputs["weight_cols"]).astype(np.int64)
    bv = np.asarray(inputs["bias_vals"], dtype=np.float32)
    bi = np.asarray(inputs["bias_idx"]).astype(np.int64)
    e0v = np.asarray(inputs["embed0_vals"], dtype=np.float32)
    e0p = np.asarray(inputs["embed0_parents"]).astype(np.int64)
    e1v = np.asarray(inputs["embed1_vals"], dtype=np.float32)
    e1p = np.asarray(inputs["embed1_parents"]).astype(np.int64)

    # dense W^T [IN_TOT, OUT] (coalesce: duplicates sum)
    wt_full = np.bincount(wc * OUT + wr, weights=wv,
                          minlength=IN_TOT * OUT).reshape(IN_TOT, OUT)

    # resolve embed parent chains to (row-in-x, multiplier), then fold the
    # expanded-feature rows of W^T into their parent rows
    q = np.empty(2 * N_EMBED, dtype=np.int64)
    a = np.empty(2 * N_EMBED, dtype=np.float64)
    q[:N_EMBED] = e0p
    a[:N_EMBED] = e0v
    for j in range(N_EMBED):
        p = int(e1p[j])
        if p < IN_BASE:
            q[N_EMBED + j] = p
            a[N_EMBED + j] = e1v[j]
        else:
            t = p - IN_BASE
            q[N_EMBED + j] = e0p[t]
            a[N_EMBED + j] = e1v[j] * e0v[t]
    wt_eff = wt_full[:IN_BASE]
    np.add.at(wt_eff, q, a[:, None] * wt_full[IN_BASE:])
    wt_eff = wt_eff.astype(np.float32)  # [2048 k, 2048 n]

    # host-side Strassen B combinations, stacked in TERM order
    B11 = wt_eff[:NB, :NB]
    B12 = wt_eff[:NB, NB:]
    B21 = wt_eff[NB:, :NB]
    B22 = wt_eff[NB:, NB:]
    bs = np.concatenate([
        B12 - B22,   # B3
        B21 - B11,   # B4
        B11,         # B2
        B22,         # B5
        B11 + B22,   # B1
        B11 + B12,   # B6
        B21 + B22,   # B7
    ], axis=0)
    wt_bf = np.ascontiguousarray(bs.astype(ml_dtypes.bfloat16))

    b = np.bincount(bi, weights=bv, minlength=OUT).astype(np.float32)
    bias_row = np.ascontiguousarray(np.broadcast_to(b[None, :], (128, OUT)))

    x_bf = x.astype(ml_dtypes.bfloat16)
    xts = [np.ascontiguousarray(x_bf[i * B_CORE:(i + 1) * B_CORE].T)
           for i in range(N_CORES)]
    return xts, wt_bf, bias_row


def kernel(**inputs) -> np.ndarray:
    import time
    from concourse.bass_utils import run_bass_kernel_spmd

    if "nc" not in _CACHED:
        _CACHED["nc"] = _build_nc()
    nc = _CACHED["nc"]

    xts, wt_bf, bias_row = _host_prep(inputs)
    in_maps = [dict(xt=xts[i], wt=wt_bf, bias=bias_row)
               for i in range(N_CORES)]
    res = None
    last_exc = None
    for attempt in range(3):
        try:
            res = run_bass_kernel_spmd(nc, in_maps,
                                       core_ids=list(range(N_CORES)))
            break
        except Exception as e:  # transient device/runtime hiccups
            last_exc = e
            time.sleep(2.0)
    if res is None:
        raise last_exc
    out = np.concatenate([res.results[i]["out"] for i in range(N_CORES)],
                         axis=0)
    return np.ascontiguousarray(out.astype(np.float32))
